# revision 35
# baseline (speedup 1.0000x reference)
"""GAT layer (dense adjacency) on 8 Trainium2 NeuronCores.

Problem: H = elu(softmax_j(mask(A, leaky_relu(Wh1_i + Wh2_j))) @ Wh),
A: [8, 2048, 2048] 0/1 f32, X: [8, 2048, 64], Ws: [64, 64], a: [128, 1].

Sharding: data-parallel over batch B=8 -> one batch element per core.

Per-core device algorithm (layout: rows = source j, cols = destination i):
  - Host precomputes Wh = X@Ws, Wh1 = Wh@a1, Wh2 = Wh@a2 (tiny) and packs
    A as fp8-e4m3 slabs with embedded bias rows.
  - Logits built by fp8 DoubleRow matmuls (0.5 cyc/row), one [128,128]
    chunk per matmul with TWO k-slots:
      slot0: lhsT = A-block [i,j],  rhs = C*I      -> C*A^T  (mask+transpose)
      slot1: lhsT = bias rows,      rhs = wh1/ones -> Wh1_i + (Wh2_j - C)
    so PSUM gets  z - C*(1-A)  directly (z = Wh1_i + Wh2_j), C = 192.
  - leaky_relu in ONE op on DVE/Pool: e = (pp * 0.2) max pp
    (scalar_tensor_tensor); masked entries stay ~ z - C -> exp ~= 0.
  - exp on ACT only, wide instructions: pa = Exp(e - S) -> fp16.
  - H^T[d, i] (+ row sums s_i via an appended ones column) accumulates on
    the tensor engine in fp16 into a [65, 2048] PSUM tile.
  - Device ships Hpre = [65, 2048] (numerators + sums); the host does the
    final divide + elu + transpose (1M elements, ~0.01% of the FLOPs).
"""
import sys

for _p in ("/opt/trn_rl_repo",):
    if _p not in sys.path:
        sys.path.append(_p)

import numpy as np
import ml_dtypes

import concourse.bass as bass
import concourse.bacc as bacc
import concourse.tile as tile
from concourse import mybir
from concourse import bass_utils

F32 = mybir.dt.float32
FP16 = mybir.dt.float16
F8 = mybir.dt.float8e4
AF = mybir.ActivationFunctionType
ALU = mybir.AluOpType
DR = mybir.MatmulPerfMode.DoubleRow
E4 = ml_dtypes.float8_e4m3

B, N, F, D = 8, 2048, 64, 64
NT = N // 128            # 16 j-tiles
C_MASK = 192.0           # mask offset; exactly representable in e4m3
ALPHA = 0.2
NW1, NW2 = 4, 6          # fp8 split counts for Wh1 / (Wh2 - C)
SLAB_W = (NT + 1) * 128  # 17 blocks of 128: 16 A blocks + 1 bias block
# exp group sizes in j-tiles per half: small first group (pipeline spin-up)
# and last group (short tail).
EGROUPS = [2, 4, 4, 4, 2]
# leaky-relu: hardware allows only ONE PSUM source per vector op and no
# GPSIMD access to PSUM at all, so the options per [128, 512] chunk are:
#   'A': ACT Prelu straight from PSUM (1 op, ~570 ns, shares ACT with exp)
#   'D': DVE 2-op (mult to SBUF scratch, then max(psum, sbuf)) ~1316 ns
#   'P': DVE fp16 copy (~658) + Pool 2-op in SBUF (~1612 ns)
# Shares 21/18/25 balance ACT(+exp)/DVE/Pool busy; smooth-interleaved so the
# in-order PE fill stream matches consumption order.
def _mk_kinds():
    shares = {"A": 31 / 64, "D": 33 / 64}
    acc = {k: 0.0 for k in shares}
    out = []
    for _ in range(64):
        for k in shares:
            acc[k] += shares[k]
        pick = max(acc, key=lambda k: acc[k])
        acc[pick] -= 1.0
        out.append(pick)
    return out


_KINDS = _mk_kinds()

_CACHED = {}


def _build_program():
    nc = bacc.Bacc("TRN2", target_bir_lowering=False, debug=False)

    ap_d = nc.dram_tensor("Apack", [NT * 128, SLAB_W], F8, kind="ExternalInput")
    rt_d = nc.dram_tensor("rhstab", [128, SLAB_W], F8, kind="ExternalInput")
    wh_d = nc.dram_tensor("whaugP", [128, NT * (D + 1)], FP16, kind="ExternalInput")
    ns_d = nc.dram_tensor("negS", [128, 1], F32, kind="ExternalInput")
    H_d = nc.dram_tensor("Hpre", [D + 1, N], F32, kind="ExternalOutput")

    with tile.TileContext(nc) as tc:
        with tc.tile_pool(name="const", bufs=1) as cp, \
             tc.tile_pool(name="aslab", bufs=NT) as apool, \
             tc.tile_pool(name="work", bufs=2) as wp, \
             tc.tile_pool(name="psA", bufs=2, space="PSUM") as psA, \
             tc.tile_pool(name="psD", bufs=2, space="PSUM") as psD, \
             tc.tile_pool(name="psG", bufs=2, space="PSUM") as psG, \
             tc.tile_pool(name="psH", bufs=1, space="PSUM") as psH:

            # ---- inputs. Parallel queues so the first fill isn't gated on a
            # serial SP DMA stream: slab0 on SP, rhstab on ACT's HWDGE, the
            # small negS/whaug on DVE's. Remaining slabs stream on SP. ----
            rhstab = cp.tile([128, SLAB_W], F8, name="rhstab")
            aslabs = [apool.tile([128, SLAB_W], F8, name=f"aslab{t}",
                                 tag="aslab") for t in range(NT)]
            negS = cp.tile([128, 1], F32, name="negS")
            whaug = cp.tile([128, NT * (D + 1)], FP16, name="whaug")
            # slab0 in three pieces so the first fill (A block 0 + bias
            # block) can start as early as possible
            nc.sync.dma_start(aslabs[0][:, 0:128], ap_d.ap()[0:128, 0:128])
            nc.sync.dma_start(aslabs[0][:, NT * 128:SLAB_W],
                              ap_d.ap()[0:128, NT * 128:SLAB_W])
            nc.scalar.dma_start(rhstab[:], rt_d.ap())
            nc.scalar.dma_start(negS[:], ns_d.ap())
            nc.sync.dma_start(aslabs[0][:, 128:NT * 128],
                              ap_d.ap()[0:128, 128:NT * 128])
            nc.scalar.dma_start(whaug[:], wh_d.ap())
            for t in range(1, NT):
                nc.sync.dma_start(aslabs[t][:],
                                  ap_d.ap()[128 * t:128 * (t + 1), :])
            # preload the Exp/Prelu table during input DMA
            warm = cp.tile([1, 1], F32, name="warm")
            nc.vector.memset(warm[:], 0.0)
            warm2 = cp.tile([1, 1], F32, name="warm2")
            nc.scalar.activation(warm2[:], warm[:], AF.Exp, bias=0.0, scale=1.0)
            alpha02 = cp.tile([128, 1], F32, name="alpha02")
            nc.vector.memset(alpha02[:], ALPHA)

            rh3 = rhstab[:].rearrange("p (x q) -> p x q", q=128)

            for h in range(2):
                ht = psH.tile([D + 1, 1024], F32, name="ht", tag="ht")
                pending_ht = []

                def emit_ht(group, ht=ht):
                    tstart, gsz, pa_t = group
                    for tt in range(tstart, tstart + gsz):
                        rel = 1024 * (tt - tstart)
                        for b2 in range(2):
                            nc.tensor.matmul(
                                ht[:, 512 * b2:512 * (b2 + 1)],
                                whaug[:, (D + 1) * tt:(D + 1) * (tt + 1)],
                                pa_t[:, rel + 512 * b2:rel + 512 * (b2 + 1)],
                                start=(tt == 0), stop=(tt == NT - 1),
                                skip_group_check=True)

                tstart = 0
                for gsz in EGROUPS:
                    e_t = wp.tile([128, 1024 * gsz], F32, name=f"e{gsz}",
                                  tag=f"e{gsz}", bufs=3)
                    pa_t = wp.tile([128, 1024 * gsz], FP16, name=f"pa{gsz}",
                                   tag=f"pa{gsz}", bufs=3)
                    for tt in range(tstart, tstart + gsz):
                        as3 = aslabs[tt][:].rearrange("p (x q) -> p x q", q=128)
                        for k in range(2):
                            g = 32 * h + 2 * tt + k
                            kind = _KINDS[g]
                            pool = {"A": psA, "D": psD, "P": psG}[kind]
                            pp = pool.tile([128, 512], F32, name="pp",
                                           tag="pp" + kind)
                            for c4 in range(4):
                                c = 8 * h + 4 * k + c4
                                nc.tensor.matmul(
                                    pp[:, 128 * c4:128 * (c4 + 1)],
                                    as3[:, c:NT + 1:NT - c, :],
                                    rh3[:, 0:2 + c:1 + c, :],
                                    start=(c4 == 0), stop=(c4 == 3),
                                    perf_mode=DR, skip_group_check=True)
                            off = 1024 * (tt - tstart) + 512 * k
                            ev = e_t[:, off:off + 512]
                            if kind == "A":
                                nc.scalar.activation(
                                    ev, pp[:], AF.Prelu, bias=0.0,
                                    scale=1.0, alpha=alpha02[:])
                            elif kind == "D":
                                u = wp.tile([128, 512], F32, name="uD",
                                            tag="uD", bufs=3)
                                nc.vector.tensor_scalar(
                                    u[:], pp[:], ALPHA, None, ALU.mult)
                                nc.vector.tensor_tensor(
                                    ev, pp[:], u[:], ALU.max)
                            else:
                                u16 = wp.tile([128, 512], FP16, name="uP",
                                              tag="uP", bufs=3)
                                nc.vector.tensor_copy(u16[:], pp[:])
                                v16 = wp.tile([128, 512], FP16, name="vP",
                                              tag="vP", bufs=3)
                                nc.gpsimd.tensor_scalar(
                                    v16[:], u16[:], ALPHA, None, ALU.mult)
                                nc.gpsimd.tensor_tensor(
                                    ev, u16[:], v16[:], ALU.max)
                    nc.scalar.activation(pa_t[:], e_t[:], AF.Exp,
                                         bias=negS[:], scale=1.0)
                    pending_ht.append((tstart, gsz, pa_t))
                    if len(pending_ht) > 2:
                        emit_ht(pending_ht.pop(0))
                    tstart += gsz
                for grp in pending_ht:
                    emit_ht(grp)

                # ship this half's numerators + sums (divide/elu on host)
                hs = wp.tile([D + 1, 1024], F32, name="hs", tag="hs", bufs=2)
                nc.vector.tensor_copy(hs[:], ht[:])
                nc.sync.dma_start(
                    H_d.ap()[:, 1024 * h:1024 * (h + 1)], hs[:])

    nc.compile()
    return nc


def _get_program():
    if "nc" not in _CACHED:
        _CACHED["nc"] = _build_program()
    return _CACHED["nc"]


def _split_e4m3(v, n):
    """Greedy hi->lo fp8-e4m3 decomposition of v (f64)."""
    out = []
    r = np.asarray(v, np.float64).copy()
    for _ in range(n):
        s = r.astype(E4).astype(np.float64)
        out.append(s)
        r -= s
    return np.stack(out)


def _host_prep(A, X, Ws, a):
    f64 = np.float64
    in_maps = []
    shifts = []
    for b in range(B):
        Wh = X[b].astype(f64) @ Ws.astype(f64)            # [N, D]
        Wh1 = (Wh @ a[:D].astype(f64))[:, 0]              # [N]
        Wh2 = (Wh @ a[D:].astype(f64))[:, 0]              # [N]
        S = max(0.0, float(Wh1.max() + Wh2.max()) - 10.5)
        shifts.append(S)
        wh1s = _split_e4m3(Wh1, NW1)                      # [NW1, N]
        wh2s = _split_e4m3(Wh2 - C_MASK, NW2)             # [NW2, N]

        # Apack[t][k, 128c+m] = A[128c+k, 128t+m] (A block x=c) plus a bias
        # block at x=16. 1.0 -> e4m3 byte 0x38 (cheap uint8 path).
        Au8 = (A[b] != 0).astype(np.uint8) * np.uint8(0x38)
        At = np.ascontiguousarray(
            Au8.reshape(NT, 128, NT, 128).transpose(2, 1, 0, 3)
        ).reshape(NT, 128, NT * 128)                      # [t, k, 2048]
        biasb = np.zeros((NT, 128, 128), np.float32)
        biasb[:, 0:NW1, :] = 1.0
        for r in range(NW2):
            biasb[:, NW1 + r, :] = wh2s[r].reshape(NT, 128)
        bias8 = biasb.astype(E4).view(np.uint8)
        apack = np.concatenate([At, bias8], axis=2).reshape(NT * 128, SLAB_W)
        apack = apack.view(E4)

        # rhstab: x=0 -> C*I; x=1+c -> rows 0..NW1-1 = wh1 splits, then ones
        rt = np.zeros((128, NT + 1, 128), np.float32)
        rt[:, 0, :] = C_MASK * np.eye(128, dtype=np.float32)
        w1r = wh1s.reshape(NW1, NT, 128)                  # [r, c, n]
        for c in range(NT):
            rt[0:NW1, 1 + c, :] = w1r[:, c, :]
            rt[NW1:NW1 + NW2, 1 + c, :] = 1.0
        rhstab = rt.reshape(128, SLAB_W).astype(E4)

        whaugP = np.ones((128, NT, D + 1), np.float16)
        whaugP[:, :, :D] = Wh.reshape(NT, 128, D).transpose(1, 0, 2)
        in_maps.append({
            "Apack": np.ascontiguousarray(apack),
            "rhstab": rhstab,
            "whaugP": np.ascontiguousarray(whaugP.reshape(128, NT * (D + 1))),
            "negS": np.full((128, 1), -S, np.float32),
        })
    return in_maps, shifts


def kernel(A, X, Ws, a, _trace=False, _trace_kwargs=None):
    A = np.asarray(A, np.float32)
    X = np.asarray(X, np.float32)
    Ws = np.asarray(Ws, np.float32)
    a = np.asarray(a, np.float32)
    nc = _get_program()
    in_maps, _shifts = _host_prep(A, X, Ws, a)
    kw = {}
    if _trace:
        kw = {"trace": True, **(_trace_kwargs or {})}
    res = bass_utils.run_bass_kernel_spmd(nc, in_maps, core_ids=list(range(B)), **kw)
    Hs = []
    for b in range(B):
        Hpre = np.asarray(res.results[b]["Hpre"]).astype(np.float64)
        num = Hpre[:D, :]                  # [D, N] numerators (transposed)
        s = Hpre[D, :]                     # [N] softmax denominators
        Hn = (num / s).T                   # [N, D]
        H = np.where(Hn > 0, Hn, np.expm1(Hn))
        Hs.append(H.astype(np.float32))
    if _trace:
        kernel.last_results = res
    return np.stack(Hs)
